# revision 34
# baseline (speedup 1.0000x reference)
"""Trainium2 Bass kernel for nn_LocationSlayerRandom (SLAYER two-branch spiking net).

Contract: kernel(**inputs) takes the FULL unsharded inputs
  spike_input [32,156,1,1,2048] f32, W1 [512,156], W2 [20,512],
  Wl1 [512,2048], Wl2 [20,512], perm [156] i32
and returns the FULL output [32,20,1,1,2204] f32.

Strategy (8 cores, data-parallel over batch, 4 samples/core):

Branch 1 (per sample b):  u1 = psp_t(W1 @ si) = W1 @ psp_t(si)
  - psp_t(si): 5 DVE scans into one 5-plane fp8 tile psAB [128, 5, T]
    (planes 0-3 = channels 0:127 per sample, plane 4 = channels 128:155 of
    all 4 samples packed at partition offsets 32b).
  - fc1 as fp8 DoubleRow: one pass contracts plane b + plane 4 (256 rows)
    against host-stacked weights (plane 1 zero-masked per sample), full
    157 TF/s rate. Thresholds on ACT as Sign(u1-10) -> fp8 {-1,0,1}; fc2
    weights pre-scaled 0.5 and the affine 0.5*rowsum(W2) correction folded
    into the threshold: for t<512 a bf16 T2 tile (DVE compare), for t>=512
    the fold is time-constant to <1e-6 so it becomes a per-partition ACT
    bias. fc2-b1 packs the 4 samples into the 4 PE column groups
    (tile_position=(0,32b)), quartered along t with chained vs scans and
    per-chunk output DMA to shrink the tail.

Branch 2: ul1 = psp_c'(Wl1 @ x_tp), x_tp host-gathered+transposed (layout
  only). A1 on PE in fp8 DoubleRow (plane-strided k-pairs), psp_c' as a
  free-dim DVE scan with per-sample reset pattern, l1 = Sign(ul1-10) on
  ACT, fc2-b2 in fp8 with 0.5-scaled Wl2 and a bf16 T2b compare (the fold
  never saturates within a 156-sample so it stays a full tile).

Numerics: all matmuls fp8 (f32 accumulate). True layer-2 potentials sit
>=7.0 (branch 1) / >=8.2 (branch 2) BELOW their thresholds under this
quantization, so layer-1 bit flips from fp8 cannot flip any output bit.
Outputs are fp8 encodings decoded on host (Sign regions via >=0).
"""

from contextlib import ExitStack

import numpy as np
import ml_dtypes

import concourse.bass as bass
import concourse.mybir as mybir
from concourse import bacc
from concourse import tile as tile_mod
from concourse.bass_utils import run_bass_kernel_spmd

F32 = mybir.dt.float32
BF16 = mybir.dt.bfloat16
FP8 = mybir.dt.float8e4
AL = mybir.AluOpType
AF = mybir.ActivationFunctionType
DR = mybir.MatmulPerfMode.DoubleRow
BF16_NP = ml_dtypes.bfloat16
FP8_NP = ml_dtypes.float8_e4m3

B, C_IN, T = 32, 156, 2048
HID, OUT_DIM = 512, 20
CP = 156                      # permuted taxel axis (branch-2 "time")
N_CORES = 8
B_PER = B // N_CORES          # 4 samples per core
ALPHA = float(np.exp(-1.0 / 10.0))
THETA = 10.0
NB2 = B_PER * CP              # 624, branch-2 packed free dim
KT = T // 128                 # 16 k-tiles over t


def build_program(tc, outs, ins):
    nc = tc.nc
    out = outs["out"]

    with ExitStack() as ctx:
        consts = ctx.enter_context(tc.tile_pool(name="consts", bufs=1))
        work = ctx.enter_context(tc.tile_pool(name="work", bufs=1))
        sgp = ctx.enter_context(tc.tile_pool(name="sgp", bufs=4))
        mid = ctx.enter_context(tc.tile_pool(name="mid", bufs=8))
        psA = ctx.enter_context(tc.tile_pool(name="psA", bufs=2, space="PSUM"))
        psF = ctx.enter_context(tc.tile_pool(name="psF", bufs=2, space="PSUM"))

        # ---------------- constant patterns (gpsimd; SBUF only) ----------
        alpha_t = consts.tile([128, T], F32, tag="alpha")
        nc.gpsimd.memset(alpha_t[:], ALPHA)
        pat624 = consts.tile([128, NB2], F32, tag="pat624")
        nc.gpsimd.memset(pat624[:], ALPHA)
        for j in range(B_PER):
            nc.gpsimd.memset(pat624[:, j * CP:j * CP + 1], 0.0)
        bias_m10 = consts.tile([128, 1], F32, tag="bm10")
        nc.gpsimd.memset(bias_m10[:], -THETA)
        act_warm = consts.tile([128, 1], F32, tag="actwarm")
        nc.scalar.activation(act_warm[:], bias_m10[:], AF.Sign,
                             bias=bias_m10[:])

        # ---------------- inputs (priority-ordered DMAs) -----------------
        siA = consts.tile([128, B_PER * T], FP8, tag="siA")
        siB = consts.tile([128, T], FP8, tag="siB")
        nc.sync.dma_start(siA[:, 0:T], ins["siA"][:, 0:T])
        nc.sync.dma_start(siB[:], ins["siB"][:])
        # sip/wl1 in per-k-pair chunks so A1 pairs start as data lands
        wl1 = consts.tile([128, KT * HID], FP8, tag="wl1")
        sip = consts.tile([128, KT * NB2], FP8, tag="sip")
        for i in range(KT // 2):
            ssl = slice(2 * i * NB2, (2 * i + 2) * NB2)
            wsl = slice(2 * i * HID, (2 * i + 2) * HID)
            nc.sync.dma_start(sip[:, ssl], ins["sipT"][:, ssl])
            nc.sync.dma_start(wl1[:, wsl], ins["Wl1T"][:, wsl])
            if i == 0:
                w1s = consts.tile([128, 16 * 256], FP8, tag="w1s")
                nc.sync.dma_start(w1s[:], ins["w1s"][:])
        for b in range(1, B_PER):
            nc.sync.dma_start(siA[:, b * T:(b + 1) * T],
                              ins["siA"][:, b * T:(b + 1) * T])
        w2p = consts.tile([128, 4 * 32], FP8, tag="w2p")
        nc.sync.dma_start(w2p[:], ins["W2pT"][:])
        w2pb = consts.tile([128, 4 * 32], FP8, tag="w2pb")
        nc.sync.dma_start(w2pb[:], ins["W2pB"][:])
        t2cpb = consts.tile([128, 1], F32, tag="t2cpb")
        nc.sync.dma_start(t2cpb[:], ins["T2cPB"][:])
        wl2p = consts.tile([128, 4 * OUT_DIM], FP8, tag="wl2p")
        nc.sync.dma_start(wl2p[:], ins["Wl2pT"][:])
        t2h = consts.tile([128, 512], BF16, tag="t2h")
        nc.sync.dma_start(t2h[:], ins["T2h"][:])
        t2cp = consts.tile([128, 1], F32, tag="t2cp")
        nc.sync.dma_start(t2cp[:], ins["T2cP"][:])
        t2b = consts.tile([128, NB2], BF16, tag="t2b")
        nc.sync.dma_start(t2b[:], ins["T2b"][:])

        # ---------------- branch-1 input psp scans (DVE, fp8) ------------
        psAB = work.tile([128, 5 * T], FP8, tag="psAB")
        psAB3 = psAB[:].rearrange("p (five t) -> p five t", five=5)
        siA3 = siA[:].rearrange("p (b t) -> p b t", b=B_PER)

        scst = work.tile([128, 5], F32, tag="scst")

        def scan_plane(b, half):
            src = siB[:] if b == 4 else siA3[:, b]
            hsl = slice(half * 1024, (half + 1) * 1024)
            if half == 0:
                # stage the h0 carry in f32 (fp8 init APs wedge the DVE)
                nc.vector.tensor_tensor_scan(psAB3[:, b, hsl],
                                             alpha_t[:, hsl], src[:, hsl],
                                             0.0, AL.mult, AL.add)
                nc.vector.tensor_copy(scst[:, b:b + 1],
                                      psAB3[:, b, 1023:1024])
            else:
                nc.vector.tensor_tensor_scan(psAB3[:, b, hsl],
                                             alpha_t[:, hsl], src[:, hsl],
                                             scst[:, b:b + 1],
                                             AL.mult, AL.add)
        scan_plane(0, 0)
        scan_plane(4, 0)

        # ---------------- branch-2 A1 (fp8 DR, ki-pair granularity) ------
        wl1_3d = wl1[:].rearrange("p (k o) -> p k o", o=HID)
        sip_3d = sip[:].rearrange("p (k c) -> p k c", c=NB2)
        ul1 = []
        l1 = []
        a1_psum = {}

        def a1_pair(m, ki):
            if ki == 0:
                a1_psum[m] = psA.tile([128, 1024], F32, tag="pA",
                                      name=f"pa{m}")
            a1 = a1_psum[m][:, :NB2]
            st, sp = (ki == 0), (ki == KT // 2 - 1)
            msl = slice(m * 128, (m + 1) * 128)
            lhs = wl1_3d[:, 2 * ki:2 * ki + 2, msl]
            nc.tensor.matmul(a1[:, 0:512], lhs,
                             sip_3d[:, 2 * ki:2 * ki + 2, 0:512],
                             start=st, stop=sp, perf_mode=DR)
            nc.tensor.matmul(a1[:, 512:NB2], lhs,
                             sip_3d[:, 2 * ki:2 * ki + 2, 512:NB2],
                             start=st, stop=sp, perf_mode=DR)
            if ki == KT // 2 - 1:
                u = mid.tile([128, NB2], F32, tag="ul1", name=f"ul1{m}")
                nc.vector.tensor_tensor_scan(u[:], pat624[:], a1, 0.0,
                                             AL.mult, AL.add)
                ul1.append(u)

        def l1_thresh(m):
            lt = mid.tile([128, NB2], FP8, tag="l1", name=f"l1{m}")
            nc.scalar.activation(lt[:], ul1[m][:], AF.Sign, bias=bias_m10[:])
            l1.append(lt)

        # ---------------- branch-1 fc1 (fp8 DR) + ACT thresholds ---------
        w1s4 = w1s[:].rearrange("p (bm two h) -> p bm two h", two=2, h=128)
        sg = {}
        for b in range(B_PER):
            sgt = sgp.tile([128, 4 * T], FP8, tag="sg", name=f"sg{b}")
            sg[b] = sgt[:].rearrange("p (m t) -> p m t", m=4)

        def fc1_unit(b, m, half):
            rhs_planes = psAB3[:, b:5:4 - b]
            lhsT = w1s4[:, b * 4 + m]
            pu = psF.tile([128, 1024], F32, tag="pF", name=f"pu{b}{m}{half}")
            for ch in range(2):
                tsl = slice(half * 1024 + ch * 512,
                            half * 1024 + (ch + 1) * 512)
                nc.tensor.matmul(pu[:, ch * 512:(ch + 1) * 512],
                                 lhsT, rhs_planes[:, :, tsl],
                                 start=True, stop=True, perf_mode=DR)
            hsl = slice(half * 1024, (half + 1) * 1024)
            if m == 3 and half == 1:
                # DVE {0,1} encode: pairs with the unscaled k3 of W2pB and
                # a fold that excludes k3 (DVE has idle here; ACT is the
                # tail-gating engine)
                nc.vector.tensor_scalar(sg[b][:, m, hsl], pu[:], THETA,
                                        None, AL.is_ge)
            else:
                nc.scalar.activation(sg[b][:, m, hsl], pu[:], AF.Sign,
                                     bias=bias_m10[:])

        # ---------------- branch 1 fc2 quarter (emitted interleaved) -----
        vs = work.tile([128, T], F32, tag="vs")
        o1 = work.tile([128, T], FP8, tag="o1")

        def fc2b1_q(q):
            qsl = slice(q * 512, (q + 1) * 512)
            puf = psF.tile([128, 1024], F32, tag="pF", name=f"pu2{q}")
            puq = puf[:, 0:512]
            wq = w2p if q < 2 else w2pb
            for k in range(4):
                ksl = slice(k * 32, k * 32 + 32)
                for b in range(B_PER):
                    nc.tensor.matmul(puq[32 * b:32 * b + 32, :],
                                     wq[:, ksl], sg[b][:, k, qsl],
                                     start=(k == 0), stop=(k == 3),
                                     tile_position=(0, 32 * b),
                                     skip_group_check=True)
            init = 0.0 if q == 0 else vs[:, q * 512 - 1:q * 512]
            nc.vector.tensor_tensor_scan(vs[:, qsl], alpha_t[:, 0:512],
                                         puq[:], init, AL.mult, AL.add)
            if q == 0:
                nc.vector.tensor_tensor(o1[:, qsl], vs[:, qsl], t2h[:],
                                        AL.is_ge)
            else:
                tq = t2cp if q == 1 else t2cpb
                nc.vector.tensor_scalar(o1[:, qsl], vs[:, qsl], tq[:],
                                        None, AL.is_ge)
            nc.sync.dma_start(
                out[:, :, qsl].rearrange("b j t -> (b j) t"), o1[:, qsl])

        def fc2b2():
            pl2f = psA.tile([128, 1024], F32, tag="pA", name="pl2")
            pl2 = pl2f[:OUT_DIM, :NB2]
            for k in range(4):
                st, sp = (k == 0), (k == 3)
                ksl = slice(k * OUT_DIM, (k + 1) * OUT_DIM)
                nc.tensor.matmul(pl2[:, 0:512], wl2p[:, ksl],
                                 l1[k][:, 0:512], start=st, stop=sp)
                nc.tensor.matmul(pl2[:, 512:NB2], wl2p[:, ksl],
                                 l1[k][:, 512:NB2], start=st, stop=sp)
            ul2 = mid.tile([128, NB2], F32, tag="ul2")
            nc.vector.tensor_tensor_scan(ul2[:OUT_DIM], pat624[:OUT_DIM],
                                         pl2, 0.0, AL.mult, AL.add)
            o2 = mid.tile([128, NB2], FP8, tag="o2")
            nc.vector.tensor_tensor(o2[:OUT_DIM], ul2[:OUT_DIM],
                                    t2b[:OUT_DIM], AL.is_ge)
            nc.sync.dma_start(
                out[:, :OUT_DIM, T:T + CP].rearrange("b o c -> o b c"),
                o2[:OUT_DIM, :].rearrange("o (b c) -> o b c", c=CP))

        # Emission: h0-major fc1 units (so fc2-b1 q0/q1 unblock mid-kernel)
        # with A1 ki-pairs interleaved one per unit to keep PE streaming
        # without starving the ACT threshold drain.
        pairs = [(m, ki) for m in range(4) for ki in range(KT // 2)]
        pi = 0

        l1_pending = []

        def a1_next(n):
            nonlocal pi
            for _ in range(n):
                if pi < len(pairs):
                    m, ki = pairs[pi]
                    a1_pair(m, ki)
                    if ki == KT // 2 - 1:
                        l1_pending.append(m)
                    pi += 1

        a1_next(8)                      # all of A1-m0 fills the ramp, so
        per_b = {0: 2, 1: 2, 2: 1, 3: 0}    # ul1-0 never stalls the scans
        for b in range(B_PER):              # front-loaded into the early
            if b >= 1:                      # scan-bound windows
                scan_plane(b, 0)            # h0 of all samples first: the
            for m in range(4):              # threshold stream never waits
                fc1_unit(b, m, 0)           # more than one scan chunk
                a1_next(per_b[b])
        scan_plane(0, 1)
        scan_plane(4, 1)
        a1_next(99)
        for b in range(B_PER):
            if b >= 1:
                scan_plane(b, 1)
            if b == 3:
                for m in l1_pending:
                    l1_thresh(m)
            for m in range(4):
                fc1_unit(b, m, 1)
            # q0/q1 after the b-group: their vs scans have ~8us of carry
            # slack, while the m3 DVE drains they'd otherwise sit ahead of
            # gate the next sample's matmuls
            if b == 2:
                fc2b1_q(0)
            if b == 3:
                fc2b1_q(1)
        fc2b2()
        fc2b1_q(2)
        fc2b1_q(3)



# ======================= host-side preparation =======================

def prep_core_inputs(si, sip, core):
    """Per-core data tensors (pure layout/dtype prep).
    si/sip are [32,156,2048] f32 (sip already perm-gathered)."""
    sl = si[core * B_PER:(core + 1) * B_PER]          # [4,156,2048]
    siA = np.ascontiguousarray(
        sl[:, :128, :].transpose(1, 0, 2).reshape(128, B_PER * T)
    ).astype(FP8_NP)
    siB = np.zeros((128, T), dtype=FP8_NP)
    for b in range(B_PER):
        siB[32 * b:32 * b + (C_IN - 128)] = sl[b, 128:C_IN, :]
    sp = sip[core * B_PER:(core + 1) * B_PER]         # [4,156,2048]
    sipT = np.ascontiguousarray(
        sp.transpose(2, 0, 1).reshape(KT, 128, NB2)
        .transpose(1, 0, 2).reshape(128, KT * NB2)
    ).astype(FP8_NP)
    return {"siA": siA, "siB": siB, "sipT": sipT}


def prep_shared_inputs(W1, W2, Wl1, Wl2):
    """Weight layouts + threshold tensors, shared by all cores."""
    w1t = np.zeros((160, HID), dtype=np.float32)
    w1t[:C_IN] = W1.T
    # fc1 DR stacked weights [128, (b,m), 2, 128]: plane 0 = channels 0:128,
    # plane 1 = channels 128:156 masked to sample b's psB partitions.
    w1s = np.zeros((128, 16, 2, 128), dtype=FP8_NP)
    for b in range(B_PER):
        for m in range(4):
            w1s[:, b * 4 + m, 0, :] = w1t[:128, m * 128:(m + 1) * 128]
            w1s[32 * b:32 * b + 32, b * 4 + m, 1, :] = \
                w1t[128:160, m * 128:(m + 1) * 128]
    w1s = np.ascontiguousarray(w1s.reshape(128, 16 * 256))

    # fc2 weights fp8, all k scaled 0.5 (uniform ACT Sign encoding),
    # padded to 32 cols per k-tile. [128, 4*32]: [p, k*32+o]
    w2t = W2.T.astype(np.float32)                     # [512, 20]
    W2pT = np.zeros((128, 4 * 32), dtype=FP8_NP)
    for k in range(4):
        W2pT[:, k * 32:k * 32 + OUT_DIM] = \
            (0.5 * w2t[k * 128:(k + 1) * 128]).astype(FP8_NP)
    w2_eff = np.empty((HID, OUT_DIM), dtype=np.float32)
    for k in range(4):
        w2_eff[k * 128:(k + 1) * 128] = \
            2.0 * W2pT[:, k * 32:k * 32 + OUT_DIM].astype(np.float32)
    r2 = w2_eff.sum(axis=0)                           # [20]
    # q2/q3 weights: k0-2 as W2pT (Sign encode), k3 unscaled ({0,1} encode
    # from the DVE threshold); fold excludes k3
    W2pB = W2pT.copy()
    W2pB[:, 3 * 32:3 * 32 + OUT_DIM] = w2t[384:512].astype(FP8_NP)
    r2_012 = w2_eff[:384].sum(axis=0)                 # [20]
    g = (1.0 - ALPHA ** (np.arange(T, dtype=np.float64) + 1)) / (1.0 - ALPHA)
    theta2 = (THETA - 0.5 * np.outer(r2, g)).astype(np.float32)   # [20, T]
    T2h = np.full((128, 512), 3e38, dtype=BF16_NP)
    T2cP = np.full((128, 1), 3e38, dtype=np.float32)
    T2cPB = np.full((128, 1), 3e38, dtype=np.float32)
    ginf = 1.0 / (1.0 - ALPHA)
    for b in range(B_PER):
        T2h[32 * b:32 * b + OUT_DIM] = theta2[:, :512]
        T2cP[32 * b:32 * b + OUT_DIM, 0] = THETA - 0.5 * r2 * ginf
        T2cPB[32 * b:32 * b + OUT_DIM, 0] = THETA - 0.5 * r2_012 * ginf

    # branch-2: Wl2 fp8 scaled 0.5 (Sign-encoded l1) + T2b fold tile.
    wl2t = Wl2.T.astype(np.float32)                   # [512, 20]
    Wl2pT = np.zeros((128, 4 * OUT_DIM), dtype=FP8_NP)
    for k in range(4):
        Wl2pT[:, k * OUT_DIM:(k + 1) * OUT_DIM] = \
            (0.5 * wl2t[k * 128:(k + 1) * 128]).astype(FP8_NP)
    wl2_eff = np.empty((HID, OUT_DIM), dtype=np.float32)
    for k in range(4):
        wl2_eff[k * 128:(k + 1) * 128] = \
            2.0 * Wl2pT[:, k * OUT_DIM:(k + 1) * OUT_DIM].astype(np.float32)
    rl2 = wl2_eff.sum(axis=0)
    gc = (1.0 - ALPHA ** (np.arange(CP, dtype=np.float64) + 1)) / (1.0 - ALPHA)
    t2b_pat = (THETA - 0.5 * np.outer(rl2, gc)).astype(np.float32)  # [20,156]
    T2b = np.full((128, NB2), 3e38, dtype=BF16_NP)
    T2b[:OUT_DIM] = np.tile(t2b_pat, (1, B_PER))

    # Wl1T [128, KT*HID]: [p, k*HID+o] = Wl1[o, 128k+p]
    Wl1T = np.ascontiguousarray(
        Wl1.T.reshape(KT, 128, HID).transpose(1, 0, 2).reshape(128, KT * HID)
    ).astype(FP8_NP)
    return {"w1s": w1s, "W2pT": W2pT, "W2pB": W2pB, "Wl2pT": Wl2pT,
            "Wl1T": Wl1T, "T2h": T2h, "T2cP": T2cP, "T2cPB": T2cPB,
            "T2b": T2b}


def make_in_maps(spike_input, W1, W2, Wl1, Wl2, perm):
    si = np.asarray(spike_input, dtype=np.float32).reshape(B, C_IN, T)
    perm = np.asarray(perm).astype(np.int64)
    sip = si[:, perm, :]                              # perm-gather (layout only)
    shared = prep_shared_inputs(np.asarray(W1, np.float32),
                                np.asarray(W2, np.float32),
                                np.asarray(Wl1, np.float32),
                                np.asarray(Wl2, np.float32))
    in_maps = []
    for core in range(N_CORES):
        m = dict(shared)
        m.update(prep_core_inputs(si, sip, core))
        in_maps.append(m)
    return in_maps


_IN_SPECS = {
    "siA": ((128, B_PER * T), FP8),
    "siB": ((128, T), FP8),
    "sipT": ((128, KT * NB2), FP8),
    "w1s": ((128, 16 * 256), FP8),
    "W2pT": ((128, 4 * 32), FP8),
    "W2pB": ((128, 4 * 32), FP8),
    "T2cPB": ((128, 1), F32),
    "Wl1T": ((128, KT * HID), FP8),
    "Wl2pT": ((128, 4 * OUT_DIM), FP8),
    "T2h": ((128, 512), BF16),
    "T2cP": ((128, 1), F32),
    "T2b": ((128, NB2), BF16),
}


def build_bass():
    nc = bacc.Bacc("TRN2", target_bir_lowering=False, debug=False)
    ins = {}
    for name, (shape, dt) in _IN_SPECS.items():
        h = nc.dram_tensor(name, list(shape), dt, kind="ExternalInput")
        ins[name] = h[:]
    out_h = nc.dram_tensor("out", [B_PER, 32, T + CP], FP8,
                           kind="ExternalOutput")
    outs = {"out": out_h[:]}
    with tile_mod.TileContext(nc) as tc:
        build_program(tc, outs, ins)
    nc.compile()
    return nc


_NC_CACHE = None


def run(inputs, trace=False, **kw):
    """Run on the 8 NeuronCores; returns (full_output, BassKernelResults)."""
    global _NC_CACHE
    if _NC_CACHE is None:
        _NC_CACHE = build_bass()
    nc = _NC_CACHE
    in_maps = make_in_maps(**inputs)
    res = run_bass_kernel_spmd(nc, in_maps, core_ids=list(range(N_CORES)),
                               trace=trace, **kw)
    parts = []
    for c in range(N_CORES):
        raw = res.results[c]["out"][:, :OUT_DIM, :].astype(np.float32)
        dec = np.empty_like(raw)
        dec[:] = (raw > 0.5)                              # all DVE is_ge {0,1}
        parts.append(dec)
    full = np.concatenate(parts, axis=0).reshape(B, OUT_DIM, 1, 1, T + CP)
    return np.ascontiguousarray(full.astype(np.float32)), res


def kernel(**inputs):
    out, _ = run(inputs)
    return out


# revision 35
# speedup vs baseline: 1.0148x; 1.0148x over previous
"""Trainium2 Bass kernel for nn_LocationSlayerRandom (SLAYER two-branch spiking net).

Contract: kernel(**inputs) takes the FULL unsharded inputs
  spike_input [32,156,1,1,2048] f32, W1 [512,156], W2 [20,512],
  Wl1 [512,2048], Wl2 [20,512], perm [156] i32
and returns the FULL output [32,20,1,1,2204] f32.

Strategy (8 cores, data-parallel over batch, 4 samples/core):

Branch 1 (per sample b):  u1 = psp_t(W1 @ si) = W1 @ psp_t(si)
  - psp_t(si): 5 DVE scans into one 5-plane fp8 tile psAB [128, 5, T]
    (planes 0-3 = channels 0:127 per sample, plane 4 = channels 128:155 of
    all 4 samples packed at partition offsets 32b).
  - fc1 as fp8 DoubleRow: one pass contracts plane b + plane 4 (256 rows)
    against host-stacked weights (plane 1 zero-masked per sample), full
    157 TF/s rate. Thresholds on ACT as Sign(u1-10) -> fp8 {-1,0,1}; fc2
    weights pre-scaled 0.5 and the affine 0.5*rowsum(W2) correction folded
    into the threshold: for t<512 a bf16 T2 tile (DVE compare), for t>=512
    the fold is time-constant to <1e-6 so it becomes a per-partition ACT
    bias. fc2-b1 packs the 4 samples into the 4 PE column groups
    (tile_position=(0,32b)), quartered along t with chained vs scans and
    per-chunk output DMA to shrink the tail.

Branch 2: ul1 = psp_c'(Wl1 @ x_tp), x_tp host-gathered+transposed (layout
  only). A1 on PE in fp8 DoubleRow (plane-strided k-pairs), psp_c' as a
  free-dim DVE scan with per-sample reset pattern, l1 = Sign(ul1-10) on
  ACT, fc2-b2 in fp8 with 0.5-scaled Wl2 and a bf16 T2b compare (the fold
  never saturates within a 156-sample so it stays a full tile).

Numerics: all matmuls fp8 (f32 accumulate). True layer-2 potentials sit
>=7.0 (branch 1) / >=8.2 (branch 2) BELOW their thresholds under this
quantization, so layer-1 bit flips from fp8 cannot flip any output bit.
Outputs are fp8 encodings decoded on host (Sign regions via >=0).
"""

from contextlib import ExitStack

import numpy as np
import ml_dtypes

import concourse.bass as bass
import concourse.mybir as mybir
from concourse import bacc
from concourse import tile as tile_mod
from concourse.bass_utils import run_bass_kernel_spmd

F32 = mybir.dt.float32
BF16 = mybir.dt.bfloat16
FP8 = mybir.dt.float8e4
AL = mybir.AluOpType
AF = mybir.ActivationFunctionType
DR = mybir.MatmulPerfMode.DoubleRow
BF16_NP = ml_dtypes.bfloat16
FP8_NP = ml_dtypes.float8_e4m3

B, C_IN, T = 32, 156, 2048
HID, OUT_DIM = 512, 20
CP = 156                      # permuted taxel axis (branch-2 "time")
N_CORES = 8
B_PER = B // N_CORES          # 4 samples per core
ALPHA = float(np.exp(-1.0 / 10.0))
THETA = 10.0
NB2 = B_PER * CP              # 624, branch-2 packed free dim
KT = T // 128                 # 16 k-tiles over t


def build_program(tc, outs, ins):
    nc = tc.nc
    out = outs["out"]

    with ExitStack() as ctx:
        consts = ctx.enter_context(tc.tile_pool(name="consts", bufs=1))
        work = ctx.enter_context(tc.tile_pool(name="work", bufs=1))
        sgp = ctx.enter_context(tc.tile_pool(name="sgp", bufs=4))
        mid = ctx.enter_context(tc.tile_pool(name="mid", bufs=8))
        psA = ctx.enter_context(tc.tile_pool(name="psA", bufs=2, space="PSUM"))
        psF = ctx.enter_context(tc.tile_pool(name="psF", bufs=2, space="PSUM"))

        # ---------------- constant patterns (gpsimd; SBUF only) ----------
        alpha_t = consts.tile([128, T], F32, tag="alpha")
        nc.gpsimd.memset(alpha_t[:], ALPHA)
        pat624 = consts.tile([128, NB2], F32, tag="pat624")
        nc.gpsimd.memset(pat624[:], ALPHA)
        for j in range(B_PER):
            nc.gpsimd.memset(pat624[:, j * CP:j * CP + 1], 0.0)
        bias_m10 = consts.tile([128, 1], F32, tag="bm10")
        nc.gpsimd.memset(bias_m10[:], -THETA)
        act_warm = consts.tile([128, 1], F32, tag="actwarm")
        nc.scalar.activation(act_warm[:], bias_m10[:], AF.Sign,
                             bias=bias_m10[:])

        # ---------------- inputs (priority-ordered DMAs) -----------------
        siA = consts.tile([128, B_PER * T], FP8, tag="siA")
        siB = consts.tile([128, T], FP8, tag="siB")
        nc.sync.dma_start(siA[:, 0:T], ins["siA"][:, 0:T])
        nc.sync.dma_start(siB[:], ins["siB"][:])
        # sip/wl1 in per-k-pair chunks so A1 pairs start as data lands
        wl1 = consts.tile([128, KT * HID], FP8, tag="wl1")
        sip = consts.tile([128, KT * NB2], FP8, tag="sip")
        for i in range(KT // 2):
            ssl = slice(2 * i * NB2, (2 * i + 2) * NB2)
            wsl = slice(2 * i * HID, (2 * i + 2) * HID)
            nc.sync.dma_start(sip[:, ssl], ins["sipT"][:, ssl])
            nc.sync.dma_start(wl1[:, wsl], ins["Wl1T"][:, wsl])
            if i == 0:
                w1s = consts.tile([128, 16 * 256], FP8, tag="w1s")
                nc.sync.dma_start(w1s[:], ins["w1s"][:])
        for b in range(1, B_PER):
            nc.sync.dma_start(siA[:, b * T:(b + 1) * T],
                              ins["siA"][:, b * T:(b + 1) * T])
        w2p = consts.tile([128, 4 * 32], FP8, tag="w2p")
        nc.sync.dma_start(w2p[:], ins["W2pT"][:])
        w2pb = consts.tile([128, 4 * 32], FP8, tag="w2pb")
        nc.sync.dma_start(w2pb[:], ins["W2pB"][:])
        t2cpb = consts.tile([128, 1], F32, tag="t2cpb")
        nc.sync.dma_start(t2cpb[:], ins["T2cPB"][:])
        wl2p = consts.tile([128, 4 * OUT_DIM], FP8, tag="wl2p")
        nc.sync.dma_start(wl2p[:], ins["Wl2pT"][:])
        t2h = consts.tile([128, 512], BF16, tag="t2h")
        nc.sync.dma_start(t2h[:], ins["T2h"][:])
        t2cp = consts.tile([128, 1], F32, tag="t2cp")
        nc.sync.dma_start(t2cp[:], ins["T2cP"][:])
        t2b = consts.tile([128, NB2], BF16, tag="t2b")
        nc.sync.dma_start(t2b[:], ins["T2b"][:])

        # ---------------- branch-1 input psp scans (DVE, fp8) ------------
        psAB = work.tile([128, 5 * T], FP8, tag="psAB")
        psAB3 = psAB[:].rearrange("p (five t) -> p five t", five=5)
        siA3 = siA[:].rearrange("p (b t) -> p b t", b=B_PER)

        scst = work.tile([128, 5], F32, tag="scst")

        def scan_plane(b, half):
            src = siB[:] if b == 4 else siA3[:, b]
            hsl = slice(half * 1024, (half + 1) * 1024)
            if half == 0:
                # stage the h0 carry in f32 (fp8 init APs wedge the DVE)
                nc.vector.tensor_tensor_scan(psAB3[:, b, hsl],
                                             alpha_t[:, hsl], src[:, hsl],
                                             0.0, AL.mult, AL.add)
                nc.vector.tensor_copy(scst[:, b:b + 1],
                                      psAB3[:, b, 1023:1024])
            else:
                nc.vector.tensor_tensor_scan(psAB3[:, b, hsl],
                                             alpha_t[:, hsl], src[:, hsl],
                                             scst[:, b:b + 1],
                                             AL.mult, AL.add)
        scan_plane(0, 0)
        scan_plane(4, 0)

        # ---------------- branch-2 A1 (fp8 DR, ki-pair granularity) ------
        wl1_3d = wl1[:].rearrange("p (k o) -> p k o", o=HID)
        sip_3d = sip[:].rearrange("p (k c) -> p k c", c=NB2)
        ul1 = []
        l1 = []
        a1_psum = {}

        def a1_pair(m, ki):
            if ki == 0:
                a1_psum[m] = psA.tile([128, 1024], F32, tag="pA",
                                      name=f"pa{m}")
            a1 = a1_psum[m][:, :NB2]
            st, sp = (ki == 0), (ki == KT // 2 - 1)
            msl = slice(m * 128, (m + 1) * 128)
            lhs = wl1_3d[:, 2 * ki:2 * ki + 2, msl]
            nc.tensor.matmul(a1[:, 0:512], lhs,
                             sip_3d[:, 2 * ki:2 * ki + 2, 0:512],
                             start=st, stop=sp, perf_mode=DR)
            nc.tensor.matmul(a1[:, 512:NB2], lhs,
                             sip_3d[:, 2 * ki:2 * ki + 2, 512:NB2],
                             start=st, stop=sp, perf_mode=DR)
            if ki == KT // 2 - 1:
                u = mid.tile([128, NB2], F32, tag="ul1", name=f"ul1{m}")
                nc.vector.tensor_tensor_scan(u[:], pat624[:], a1, 0.0,
                                             AL.mult, AL.add)
                ul1.append(u)

        def l1_thresh(m):
            lt = mid.tile([128, NB2], FP8, tag="l1", name=f"l1{m}")
            nc.scalar.activation(lt[:], ul1[m][:], AF.Sign, bias=bias_m10[:])
            l1.append(lt)

        # ---------------- branch-1 fc1 (fp8 DR) + ACT thresholds ---------
        w1s4 = w1s[:].rearrange("p (bm two h) -> p bm two h", two=2, h=128)
        sg = {}
        for b in range(B_PER):
            sgt = sgp.tile([128, 4 * T], FP8, tag="sg", name=f"sg{b}")
            sg[b] = sgt[:].rearrange("p (m t) -> p m t", m=4)

        def fc1_unit(b, m, half):
            rhs_planes = psAB3[:, b:5:4 - b]
            lhsT = w1s4[:, b * 4 + m]
            pu = psF.tile([128, 1024], F32, tag="pF", name=f"pu{b}{m}{half}")
            for ch in range(2):
                tsl = slice(half * 1024 + ch * 512,
                            half * 1024 + (ch + 1) * 512)
                nc.tensor.matmul(pu[:, ch * 512:(ch + 1) * 512],
                                 lhsT, rhs_planes[:, :, tsl],
                                 start=True, stop=True, perf_mode=DR)
            hsl = slice(half * 1024, (half + 1) * 1024)
            if m == 3 and half == 1:
                # DVE {0,1} encode: pairs with the unscaled k3 of W2pB and
                # a fold that excludes k3 (DVE has idle here; ACT is the
                # tail-gating engine)
                nc.vector.tensor_scalar(sg[b][:, m, hsl], pu[:], THETA,
                                        None, AL.is_ge)
            else:
                nc.scalar.activation(sg[b][:, m, hsl], pu[:], AF.Sign,
                                     bias=bias_m10[:])

        # ---------------- branch 1 fc2 quarter (emitted interleaved) -----
        vs = work.tile([128, T], F32, tag="vs")
        o1 = work.tile([128, T], FP8, tag="o1")

        def fc2b1_q(q):
            qsl = slice(q * 512, (q + 1) * 512)
            puf = psF.tile([128, 1024], F32, tag="pF", name=f"pu2{q}")
            puq = puf[:, 0:512]
            wq = w2p if q < 2 else w2pb
            for k in range(4):
                ksl = slice(k * 32, k * 32 + 32)
                for b in range(B_PER):
                    nc.tensor.matmul(puq[32 * b:32 * b + 32, :],
                                     wq[:, ksl], sg[b][:, k, qsl],
                                     start=(k == 0), stop=(k == 3),
                                     tile_position=(0, 32 * b),
                                     skip_group_check=True)
            init = 0.0 if q == 0 else vs[:, q * 512 - 1:q * 512]
            nc.vector.tensor_tensor_scan(vs[:, qsl], alpha_t[:, 0:512],
                                         puq[:], init, AL.mult, AL.add)
            if q == 0:
                nc.vector.tensor_tensor(o1[:, qsl], vs[:, qsl], t2h[:],
                                        AL.is_ge)
            else:
                tq = t2cp if q == 1 else t2cpb
                nc.vector.tensor_scalar(o1[:, qsl], vs[:, qsl], tq[:],
                                        None, AL.is_ge)
            nc.sync.dma_start(
                out[:, :, qsl].rearrange("b j t -> (b j) t"), o1[:, qsl])

        def fc2b2():
            pl2f = psA.tile([128, 1024], F32, tag="pA", name="pl2")
            pl2 = pl2f[:OUT_DIM, :NB2]
            for k in range(4):
                st, sp = (k == 0), (k == 3)
                ksl = slice(k * OUT_DIM, (k + 1) * OUT_DIM)
                nc.tensor.matmul(pl2[:, 0:512], wl2p[:, ksl],
                                 l1[k][:, 0:512], start=st, stop=sp)
                nc.tensor.matmul(pl2[:, 512:NB2], wl2p[:, ksl],
                                 l1[k][:, 512:NB2], start=st, stop=sp)
            ul2 = mid.tile([128, NB2], F32, tag="ul2")
            nc.vector.tensor_tensor_scan(ul2[:OUT_DIM], pat624[:OUT_DIM],
                                         pl2, 0.0, AL.mult, AL.add)
            o2 = mid.tile([128, NB2], FP8, tag="o2")
            nc.vector.tensor_tensor(o2[:OUT_DIM], ul2[:OUT_DIM],
                                    t2b[:OUT_DIM], AL.is_ge)
            nc.sync.dma_start(
                out[:, :OUT_DIM, T:T + CP].rearrange("b o c -> o b c"),
                o2[:OUT_DIM, :].rearrange("o (b c) -> o b c", c=CP))

        # Emission: h0-major fc1 units (so fc2-b1 q0/q1 unblock mid-kernel)
        # with A1 ki-pairs interleaved one per unit to keep PE streaming
        # without starving the ACT threshold drain.
        pairs = [(m, ki) for m in range(4) for ki in range(KT // 2)]
        pi = 0

        l1_pending = []

        def a1_next(n):
            nonlocal pi
            for _ in range(n):
                if pi < len(pairs):
                    m, ki = pairs[pi]
                    a1_pair(m, ki)
                    if ki == KT // 2 - 1:
                        l1_pending.append(m)
                    pi += 1

        a1_next(8)                      # all of A1-m0 fills the ramp, so
        per_b = {0: 2, 1: 2, 2: 1, 3: 0}    # ul1-0 never stalls the scans
        for b in range(B_PER):              # front-loaded into the early
            if b >= 1:                      # scan-bound windows
                scan_plane(b, 0)            # h0 of all samples first: the
            for m in range(4):              # threshold stream never waits
                fc1_unit(b, m, 0)           # more than one scan chunk
                a1_next(per_b[b])
        scan_plane(0, 1)
        scan_plane(4, 1)
        a1_next(99)
        for b in range(B_PER):
            if b >= 1:
                scan_plane(b, 1)
            if b == 2:
                fc2b1_q(0)                  # h0-only quarters slot into the
            if b == 3:
                fc2b1_q(1)                  # h1 phase once h0 thr are done
                for m in l1_pending:
                    l1_thresh(m)
            for m in range(4):
                fc1_unit(b, m, 1)
        fc2b2()
        fc2b1_q(2)
        fc2b1_q(3)



# ======================= host-side preparation =======================

def prep_core_inputs(si, sip, core):
    """Per-core data tensors (pure layout/dtype prep).
    si/sip are [32,156,2048] f32 (sip already perm-gathered)."""
    sl = si[core * B_PER:(core + 1) * B_PER]          # [4,156,2048]
    siA = np.ascontiguousarray(
        sl[:, :128, :].transpose(1, 0, 2).reshape(128, B_PER * T)
    ).astype(FP8_NP)
    siB = np.zeros((128, T), dtype=FP8_NP)
    for b in range(B_PER):
        siB[32 * b:32 * b + (C_IN - 128)] = sl[b, 128:C_IN, :]
    sp = sip[core * B_PER:(core + 1) * B_PER]         # [4,156,2048]
    sipT = np.ascontiguousarray(
        sp.transpose(2, 0, 1).reshape(KT, 128, NB2)
        .transpose(1, 0, 2).reshape(128, KT * NB2)
    ).astype(FP8_NP)
    return {"siA": siA, "siB": siB, "sipT": sipT}


def prep_shared_inputs(W1, W2, Wl1, Wl2):
    """Weight layouts + threshold tensors, shared by all cores."""
    w1t = np.zeros((160, HID), dtype=np.float32)
    w1t[:C_IN] = W1.T
    # fc1 DR stacked weights [128, (b,m), 2, 128]: plane 0 = channels 0:128,
    # plane 1 = channels 128:156 masked to sample b's psB partitions.
    w1s = np.zeros((128, 16, 2, 128), dtype=FP8_NP)
    for b in range(B_PER):
        for m in range(4):
            w1s[:, b * 4 + m, 0, :] = w1t[:128, m * 128:(m + 1) * 128]
            w1s[32 * b:32 * b + 32, b * 4 + m, 1, :] = \
                w1t[128:160, m * 128:(m + 1) * 128]
    w1s = np.ascontiguousarray(w1s.reshape(128, 16 * 256))

    # fc2 weights fp8, all k scaled 0.5 (uniform ACT Sign encoding),
    # padded to 32 cols per k-tile. [128, 4*32]: [p, k*32+o]
    w2t = W2.T.astype(np.float32)                     # [512, 20]
    W2pT = np.zeros((128, 4 * 32), dtype=FP8_NP)
    for k in range(4):
        W2pT[:, k * 32:k * 32 + OUT_DIM] = \
            (0.5 * w2t[k * 128:(k + 1) * 128]).astype(FP8_NP)
    w2_eff = np.empty((HID, OUT_DIM), dtype=np.float32)
    for k in range(4):
        w2_eff[k * 128:(k + 1) * 128] = \
            2.0 * W2pT[:, k * 32:k * 32 + OUT_DIM].astype(np.float32)
    r2 = w2_eff.sum(axis=0)                           # [20]
    # q2/q3 weights: k0-2 as W2pT (Sign encode), k3 unscaled ({0,1} encode
    # from the DVE threshold); fold excludes k3
    W2pB = W2pT.copy()
    W2pB[:, 3 * 32:3 * 32 + OUT_DIM] = w2t[384:512].astype(FP8_NP)
    r2_012 = w2_eff[:384].sum(axis=0)                 # [20]
    g = (1.0 - ALPHA ** (np.arange(T, dtype=np.float64) + 1)) / (1.0 - ALPHA)
    theta2 = (THETA - 0.5 * np.outer(r2, g)).astype(np.float32)   # [20, T]
    T2h = np.full((128, 512), 3e38, dtype=BF16_NP)
    T2cP = np.full((128, 1), 3e38, dtype=np.float32)
    T2cPB = np.full((128, 1), 3e38, dtype=np.float32)
    ginf = 1.0 / (1.0 - ALPHA)
    for b in range(B_PER):
        T2h[32 * b:32 * b + OUT_DIM] = theta2[:, :512]
        T2cP[32 * b:32 * b + OUT_DIM, 0] = THETA - 0.5 * r2 * ginf
        T2cPB[32 * b:32 * b + OUT_DIM, 0] = THETA - 0.5 * r2_012 * ginf

    # branch-2: Wl2 fp8 scaled 0.5 (Sign-encoded l1) + T2b fold tile.
    wl2t = Wl2.T.astype(np.float32)                   # [512, 20]
    Wl2pT = np.zeros((128, 4 * OUT_DIM), dtype=FP8_NP)
    for k in range(4):
        Wl2pT[:, k * OUT_DIM:(k + 1) * OUT_DIM] = \
            (0.5 * wl2t[k * 128:(k + 1) * 128]).astype(FP8_NP)
    wl2_eff = np.empty((HID, OUT_DIM), dtype=np.float32)
    for k in range(4):
        wl2_eff[k * 128:(k + 1) * 128] = \
            2.0 * Wl2pT[:, k * OUT_DIM:(k + 1) * OUT_DIM].astype(np.float32)
    rl2 = wl2_eff.sum(axis=0)
    gc = (1.0 - ALPHA ** (np.arange(CP, dtype=np.float64) + 1)) / (1.0 - ALPHA)
    t2b_pat = (THETA - 0.5 * np.outer(rl2, gc)).astype(np.float32)  # [20,156]
    T2b = np.full((128, NB2), 3e38, dtype=BF16_NP)
    T2b[:OUT_DIM] = np.tile(t2b_pat, (1, B_PER))

    # Wl1T [128, KT*HID]: [p, k*HID+o] = Wl1[o, 128k+p]
    Wl1T = np.ascontiguousarray(
        Wl1.T.reshape(KT, 128, HID).transpose(1, 0, 2).reshape(128, KT * HID)
    ).astype(FP8_NP)
    return {"w1s": w1s, "W2pT": W2pT, "W2pB": W2pB, "Wl2pT": Wl2pT,
            "Wl1T": Wl1T, "T2h": T2h, "T2cP": T2cP, "T2cPB": T2cPB,
            "T2b": T2b}


def make_in_maps(spike_input, W1, W2, Wl1, Wl2, perm):
    si = np.asarray(spike_input, dtype=np.float32).reshape(B, C_IN, T)
    perm = np.asarray(perm).astype(np.int64)
    sip = si[:, perm, :]                              # perm-gather (layout only)
    shared = prep_shared_inputs(np.asarray(W1, np.float32),
                                np.asarray(W2, np.float32),
                                np.asarray(Wl1, np.float32),
                                np.asarray(Wl2, np.float32))
    in_maps = []
    for core in range(N_CORES):
        m = dict(shared)
        m.update(prep_core_inputs(si, sip, core))
        in_maps.append(m)
    return in_maps


_IN_SPECS = {
    "siA": ((128, B_PER * T), FP8),
    "siB": ((128, T), FP8),
    "sipT": ((128, KT * NB2), FP8),
    "w1s": ((128, 16 * 256), FP8),
    "W2pT": ((128, 4 * 32), FP8),
    "W2pB": ((128, 4 * 32), FP8),
    "T2cPB": ((128, 1), F32),
    "Wl1T": ((128, KT * HID), FP8),
    "Wl2pT": ((128, 4 * OUT_DIM), FP8),
    "T2h": ((128, 512), BF16),
    "T2cP": ((128, 1), F32),
    "T2b": ((128, NB2), BF16),
}


def build_bass():
    nc = bacc.Bacc("TRN2", target_bir_lowering=False, debug=False)
    ins = {}
    for name, (shape, dt) in _IN_SPECS.items():
        h = nc.dram_tensor(name, list(shape), dt, kind="ExternalInput")
        ins[name] = h[:]
    out_h = nc.dram_tensor("out", [B_PER, 32, T + CP], FP8,
                           kind="ExternalOutput")
    outs = {"out": out_h[:]}
    with tile_mod.TileContext(nc) as tc:
        build_program(tc, outs, ins)
    nc.compile()
    return nc


_NC_CACHE = None


def run(inputs, trace=False, **kw):
    """Run on the 8 NeuronCores; returns (full_output, BassKernelResults)."""
    global _NC_CACHE
    if _NC_CACHE is None:
        _NC_CACHE = build_bass()
    nc = _NC_CACHE
    in_maps = make_in_maps(**inputs)
    res = run_bass_kernel_spmd(nc, in_maps, core_ids=list(range(N_CORES)),
                               trace=trace, **kw)
    parts = []
    for c in range(N_CORES):
        raw = res.results[c]["out"][:, :OUT_DIM, :].astype(np.float32)
        dec = np.empty_like(raw)
        dec[:] = (raw > 0.5)                              # all DVE is_ge {0,1}
        parts.append(dec)
    full = np.concatenate(parts, axis=0).reshape(B, OUT_DIM, 1, 1, T + CP)
    return np.ascontiguousarray(full.astype(np.float32)), res


def kernel(**inputs):
    out, _ = run(inputs)
    return out


# revision 36
# speedup vs baseline: 1.1156x; 1.0993x over previous
"""Trainium2 Bass kernel for nn_LocationSlayerRandom (SLAYER two-branch spiking net).

Contract: kernel(**inputs) takes the FULL unsharded inputs
  spike_input [32,156,1,1,2048] f32, W1 [512,156], W2 [20,512],
  Wl1 [512,2048], Wl2 [20,512], perm [156] i32
and returns the FULL output [32,20,1,1,2204] f32.

Strategy (8 cores, data-parallel over batch, 4 samples/core):

Branch 1 (per sample b):  u1 = psp_t(W1 @ si) = W1 @ psp_t(si)
  - psp_t(si): 5 DVE scans into one 5-plane fp8 tile psAB [128, 5, T]
    (planes 0-3 = channels 0:127 per sample, plane 4 = channels 128:155 of
    all 4 samples packed at partition offsets 32b).
  - fc1 as fp8 DoubleRow: one pass contracts plane b + plane 4 (256 rows)
    against host-stacked weights (plane 1 zero-masked per sample), full
    157 TF/s rate. Thresholds on ACT as Sign(u1-10) -> fp8 {-1,0,1}; fc2
    weights pre-scaled 0.5 and the affine 0.5*rowsum(W2) correction folded
    into the threshold: for t<512 a bf16 T2 tile (DVE compare), for t>=512
    the fold is time-constant to <1e-6 so it becomes a per-partition ACT
    bias. fc2-b1 packs the 4 samples into the 4 PE column groups
    (tile_position=(0,32b)), quartered along t with chained vs scans and
    per-chunk output DMA to shrink the tail.

Branch 2: ul1 = psp_c'(Wl1 @ x_tp), x_tp host-gathered+transposed (layout
  only). A1 on PE in fp8 DoubleRow (plane-strided k-pairs), psp_c' as a
  free-dim DVE scan with per-sample reset pattern, l1 = Sign(ul1-10) on
  ACT, fc2-b2 in fp8 with 0.5-scaled Wl2 and a bf16 T2b compare (the fold
  never saturates within a 156-sample so it stays a full tile).

Numerics: all matmuls fp8 (f32 accumulate). True layer-2 potentials sit
>=7.0 (branch 1) / >=8.2 (branch 2) BELOW their thresholds under this
quantization, so layer-1 bit flips from fp8 cannot flip any output bit.
Outputs are fp8 encodings decoded on host (Sign regions via >=0).
"""

from contextlib import ExitStack

import numpy as np
import ml_dtypes

import concourse.bass as bass
import concourse.mybir as mybir
from concourse import bacc
from concourse import tile as tile_mod
from concourse.bass_utils import run_bass_kernel_spmd

F32 = mybir.dt.float32
BF16 = mybir.dt.bfloat16
FP8 = mybir.dt.float8e4
AL = mybir.AluOpType
AF = mybir.ActivationFunctionType
DR = mybir.MatmulPerfMode.DoubleRow
BF16_NP = ml_dtypes.bfloat16
FP8_NP = ml_dtypes.float8_e4m3

B, C_IN, T = 32, 156, 2048
HID, OUT_DIM = 512, 20
CP = 156                      # permuted taxel axis (branch-2 "time")
N_CORES = 8
B_PER = B // N_CORES          # 4 samples per core
ALPHA = float(np.exp(-1.0 / 10.0))
THETA = 10.0
NB2 = B_PER * CP              # 624, branch-2 packed free dim
KT = T // 128                 # 16 k-tiles over t


def build_program(tc, outs, ins):
    nc = tc.nc
    out = outs["out"]

    with ExitStack() as ctx:
        consts = ctx.enter_context(tc.tile_pool(name="consts", bufs=1))
        work = ctx.enter_context(tc.tile_pool(name="work", bufs=1))
        sgp = ctx.enter_context(tc.tile_pool(name="sgp", bufs=4))
        mid = ctx.enter_context(tc.tile_pool(name="mid", bufs=8))
        psA = ctx.enter_context(tc.tile_pool(name="psA", bufs=2, space="PSUM"))
        psF = ctx.enter_context(tc.tile_pool(name="psF", bufs=2, space="PSUM"))

        # ---------------- constant patterns (gpsimd; SBUF only) ----------
        alpha_t = consts.tile([128, T], F32, tag="alpha")
        nc.gpsimd.memset(alpha_t[:], ALPHA)
        pat624 = consts.tile([128, NB2], F32, tag="pat624")
        nc.gpsimd.memset(pat624[:], ALPHA)
        for j in range(B_PER):
            nc.gpsimd.memset(pat624[:, j * CP:j * CP + 1], 0.0)
        bias_m10 = consts.tile([128, 1], F32, tag="bm10")
        nc.gpsimd.memset(bias_m10[:], -THETA)
        act_warm = consts.tile([128, 1], F32, tag="actwarm")
        nc.scalar.activation(act_warm[:], bias_m10[:], AF.Sign,
                             bias=bias_m10[:])

        # ---------------- inputs (priority-ordered DMAs) -----------------
        siA = consts.tile([128, B_PER * T], FP8, tag="siA")
        siB = consts.tile([128, T], FP8, tag="siB")
        nc.sync.dma_start(siA[:, 0:T], ins["siA"][:, 0:T])
        nc.sync.dma_start(siB[:], ins["siB"][:])
        # sip/wl1 in per-k-pair chunks so A1 pairs start as data lands
        wl1 = consts.tile([128, KT * HID], FP8, tag="wl1")
        sip = consts.tile([128, KT * NB2], FP8, tag="sip")
        for i in range(KT // 2):
            ssl = slice(2 * i * NB2, (2 * i + 2) * NB2)
            wsl = slice(2 * i * HID, (2 * i + 2) * HID)
            nc.sync.dma_start(sip[:, ssl], ins["sipT"][:, ssl])
            nc.sync.dma_start(wl1[:, wsl], ins["Wl1T"][:, wsl])
            if i == 0:
                w1s = consts.tile([128, 16 * 256], FP8, tag="w1s")
                nc.sync.dma_start(w1s[:], ins["w1s"][:])
        for b in range(1, B_PER):
            nc.sync.dma_start(siA[:, b * T:(b + 1) * T],
                              ins["siA"][:, b * T:(b + 1) * T])
        w2p = consts.tile([128, 4 * 32], FP8, tag="w2p")
        nc.sync.dma_start(w2p[:], ins["W2pT"][:])
        w2pb = consts.tile([128, 4 * 32], FP8, tag="w2pb")
        nc.sync.dma_start(w2pb[:], ins["W2pB"][:])
        t2cpb = consts.tile([128, 1], F32, tag="t2cpb")
        nc.sync.dma_start(t2cpb[:], ins["T2cPB"][:])
        wl2p = consts.tile([128, 4 * OUT_DIM], FP8, tag="wl2p")
        nc.sync.dma_start(wl2p[:], ins["Wl2pT"][:])
        t2h = consts.tile([128, 512], BF16, tag="t2h")
        nc.sync.dma_start(t2h[:], ins["T2h"][:])
        t2cp = consts.tile([128, 1], F32, tag="t2cp")
        nc.sync.dma_start(t2cp[:], ins["T2cP"][:])
        t2b = consts.tile([128, NB2], BF16, tag="t2b")
        nc.sync.dma_start(t2b[:], ins["T2b"][:])

        # ---------------- branch-1 input psp scans (DVE, fp8) ------------
        psAB = work.tile([128, 5 * T], FP8, tag="psAB")
        psAB3 = psAB[:].rearrange("p (five t) -> p five t", five=5)
        siA3 = siA[:].rearrange("p (b t) -> p b t", b=B_PER)

        scst = work.tile([128, 5], F32, tag="scst")

        def scan_plane(b, half):
            src = siB[:] if b == 4 else siA3[:, b]
            hsl = slice(half * 1024, (half + 1) * 1024)
            if half == 0:
                # stage the h0 carry in f32 (fp8 init APs wedge the DVE)
                nc.vector.tensor_tensor_scan(psAB3[:, b, hsl],
                                             alpha_t[:, hsl], src[:, hsl],
                                             0.0, AL.mult, AL.add)
                nc.vector.tensor_copy(scst[:, b:b + 1],
                                      psAB3[:, b, 1023:1024])
            else:
                nc.vector.tensor_tensor_scan(psAB3[:, b, hsl],
                                             alpha_t[:, hsl], src[:, hsl],
                                             scst[:, b:b + 1],
                                             AL.mult, AL.add)
        scan_plane(0, 0)
        scan_plane(4, 0)

        # ---------------- branch-2 A1 (fp8 DR, ki-pair granularity) ------
        wl1_3d = wl1[:].rearrange("p (k o) -> p k o", o=HID)
        sip_3d = sip[:].rearrange("p (k c) -> p k c", c=NB2)
        ul1 = []
        l1 = []
        a1_psum = {}

        def a1_pair(m, ki):
            if ki == 0:
                a1_psum[m] = psA.tile([128, 1024], F32, tag="pA",
                                      name=f"pa{m}")
            a1 = a1_psum[m][:, :NB2]
            st, sp = (ki == 0), (ki == KT // 2 - 1)
            msl = slice(m * 128, (m + 1) * 128)
            lhs = wl1_3d[:, 2 * ki:2 * ki + 2, msl]
            nc.tensor.matmul(a1[:, 0:512], lhs,
                             sip_3d[:, 2 * ki:2 * ki + 2, 0:512],
                             start=st, stop=sp, perf_mode=DR)
            nc.tensor.matmul(a1[:, 512:NB2], lhs,
                             sip_3d[:, 2 * ki:2 * ki + 2, 512:NB2],
                             start=st, stop=sp, perf_mode=DR)
            if ki == KT // 2 - 1:
                u = mid.tile([128, NB2], F32, tag="ul1", name=f"ul1{m}")
                nc.vector.tensor_tensor_scan(u[:], pat624[:], a1, 0.0,
                                             AL.mult, AL.add)
                ul1.append(u)

        def l1_thresh(m):
            lt = mid.tile([128, NB2], FP8, tag="l1", name=f"l1{m}")
            nc.scalar.activation(lt[:], ul1[m][:], AF.Sign, bias=bias_m10[:])
            l1.append(lt)

        # ---------------- branch-1 fc1 (fp8 DR) + ACT thresholds ---------
        w1s4 = w1s[:].rearrange("p (bm two h) -> p bm two h", two=2, h=128)
        sg = {}
        for b in range(B_PER):
            sgt = sgp.tile([128, 4 * T], FP8, tag="sg", name=f"sg{b}")
            sg[b] = sgt[:].rearrange("p (m t) -> p m t", m=4)

        def fc1_unit(b, m, half):
            rhs_planes = psAB3[:, b:5:4 - b]
            lhsT = w1s4[:, b * 4 + m]
            pu = psF.tile([128, 1024], F32, tag="pF", name=f"pu{b}{m}{half}")
            for ch in range(2):
                tsl = slice(half * 1024 + ch * 512,
                            half * 1024 + (ch + 1) * 512)
                nc.tensor.matmul(pu[:, ch * 512:(ch + 1) * 512],
                                 lhsT, rhs_planes[:, :, tsl],
                                 start=True, stop=True, perf_mode=DR)
            hsl = slice(half * 1024, (half + 1) * 1024)
            if m == 3 and half == 1:
                # DVE {0,1} encode: pairs with the unscaled k3 of W2pB and
                # a fold that excludes k3 (DVE has idle here; ACT is the
                # tail-gating engine)
                nc.vector.tensor_scalar(sg[b][:, m, hsl], pu[:], THETA,
                                        None, AL.is_ge)
            else:
                nc.scalar.activation(sg[b][:, m, hsl], pu[:], AF.Sign,
                                     bias=bias_m10[:])

        # ---------------- branch 1 fc2 quarter (emitted interleaved) -----
        vs = work.tile([128, T], F32, tag="vs")
        o1 = work.tile([128, T], FP8, tag="o1")

        def fc2b1_q(q):
            qsl = slice(q * 512, (q + 1) * 512)
            puf = psF.tile([128, 1024], F32, tag="pF", name=f"pu2{q}")
            puq = puf[:, 0:512]
            wq = w2p if q < 2 else w2pb
            for k in range(4):
                ksl = slice(k * 32, k * 32 + 32)
                for b in range(B_PER):
                    nc.tensor.matmul(puq[32 * b:32 * b + 32, :],
                                     wq[:, ksl], sg[b][:, k, qsl],
                                     start=(k == 0), stop=(k == 3),
                                     tile_position=(0, 32 * b),
                                     skip_group_check=True)
            init = 0.0 if q == 0 else vs[:, q * 512 - 1:q * 512]
            nc.vector.tensor_tensor_scan(vs[:, qsl], alpha_t[:, 0:512],
                                         puq[:], init, AL.mult, AL.add)
            if q == 0:
                nc.vector.tensor_tensor(o1[:, qsl], vs[:, qsl], t2h[:],
                                        AL.is_ge)
            else:
                tq = t2cp if q == 1 else t2cpb
                nc.vector.tensor_scalar(o1[:, qsl], vs[:, qsl], tq[:],
                                        None, AL.is_ge)
            nc.sync.dma_start(
                out[:, :, qsl].rearrange("b j t -> (b j) t"), o1[:, qsl])

        def fc2b2():
            pl2f = psA.tile([128, 1024], F32, tag="pA", name="pl2")
            pl2 = pl2f[:OUT_DIM, :NB2]
            for k in range(4):
                st, sp = (k == 0), (k == 3)
                ksl = slice(k * OUT_DIM, (k + 1) * OUT_DIM)
                nc.tensor.matmul(pl2[:, 0:512], wl2p[:, ksl],
                                 l1[k][:, 0:512], start=st, stop=sp)
                nc.tensor.matmul(pl2[:, 512:NB2], wl2p[:, ksl],
                                 l1[k][:, 512:NB2], start=st, stop=sp)
            ul2 = mid.tile([128, NB2], F32, tag="ul2")
            nc.vector.tensor_tensor_scan(ul2[:OUT_DIM], pat624[:OUT_DIM],
                                         pl2, 0.0, AL.mult, AL.add)
            o2 = mid.tile([128, NB2], FP8, tag="o2")
            nc.vector.tensor_tensor(o2[:OUT_DIM], ul2[:OUT_DIM],
                                    t2b[:OUT_DIM], AL.is_ge)
            nc.sync.dma_start(
                out[:, :OUT_DIM, T:T + CP].rearrange("b o c -> o b c"),
                o2[:OUT_DIM, :].rearrange("o (b c) -> o b c", c=CP))

        # Emission: h0-major fc1 units (so fc2-b1 q0/q1 unblock mid-kernel)
        # with A1 ki-pairs interleaved one per unit to keep PE streaming
        # without starving the ACT threshold drain.
        pairs = [(m, ki) for m in range(4) for ki in range(KT // 2)]
        pi = 0

        l1_pending = []

        def a1_next(n):
            nonlocal pi
            for _ in range(n):
                if pi < len(pairs):
                    m, ki = pairs[pi]
                    a1_pair(m, ki)
                    if ki == KT // 2 - 1:
                        l1_pending.append(m)
                    pi += 1

        a1_next(8)                      # all of A1-m0 fills the ramp, so
        per_b = {0: 2, 1: 2, 2: 1, 3: 0}    # ul1-0 never stalls the scans
        for b in range(B_PER):              # front-loaded into the early
            if b >= 1:                      # scan-bound windows
                scan_plane(b, 0)            # h0 of all samples first: the
            for m in range(4):              # threshold stream never waits
                fc1_unit(b, m, 0)           # more than one scan chunk
                a1_next(per_b[b])
        scan_plane(0, 1)
        scan_plane(4, 1)
        a1_next(99)
        for b in range(B_PER):
            if b >= 1:
                scan_plane(b, 1)
            if b == 3:
                for m in l1_pending:
                    l1_thresh(m)
            for m in range(4):
                fc1_unit(b, m, 1)
            if b == 2:
                fc2b1_q(0)
            if b == 3:
                fc2b1_q(1)
        fc2b2()
        fc2b1_q(2)
        fc2b1_q(3)



# ======================= host-side preparation =======================

def prep_core_inputs(si, sip, core):
    """Per-core data tensors (pure layout/dtype prep).
    si/sip are [32,156,2048] f32 (sip already perm-gathered)."""
    sl = si[core * B_PER:(core + 1) * B_PER]          # [4,156,2048]
    siA = np.ascontiguousarray(
        sl[:, :128, :].transpose(1, 0, 2).reshape(128, B_PER * T)
    ).astype(FP8_NP)
    siB = np.zeros((128, T), dtype=FP8_NP)
    for b in range(B_PER):
        siB[32 * b:32 * b + (C_IN - 128)] = sl[b, 128:C_IN, :]
    sp = sip[core * B_PER:(core + 1) * B_PER]         # [4,156,2048]
    sipT = np.ascontiguousarray(
        sp.transpose(2, 0, 1).reshape(KT, 128, NB2)
        .transpose(1, 0, 2).reshape(128, KT * NB2)
    ).astype(FP8_NP)
    return {"siA": siA, "siB": siB, "sipT": sipT}


def prep_shared_inputs(W1, W2, Wl1, Wl2):
    """Weight layouts + threshold tensors, shared by all cores."""
    w1t = np.zeros((160, HID), dtype=np.float32)
    w1t[:C_IN] = W1.T
    # fc1 DR stacked weights [128, (b,m), 2, 128]: plane 0 = channels 0:128,
    # plane 1 = channels 128:156 masked to sample b's psB partitions.
    w1s = np.zeros((128, 16, 2, 128), dtype=FP8_NP)
    for b in range(B_PER):
        for m in range(4):
            w1s[:, b * 4 + m, 0, :] = w1t[:128, m * 128:(m + 1) * 128]
            w1s[32 * b:32 * b + 32, b * 4 + m, 1, :] = \
                w1t[128:160, m * 128:(m + 1) * 128]
    w1s = np.ascontiguousarray(w1s.reshape(128, 16 * 256))

    # fc2 weights fp8, all k scaled 0.5 (uniform ACT Sign encoding),
    # padded to 32 cols per k-tile. [128, 4*32]: [p, k*32+o]
    w2t = W2.T.astype(np.float32)                     # [512, 20]
    W2pT = np.zeros((128, 4 * 32), dtype=FP8_NP)
    for k in range(4):
        W2pT[:, k * 32:k * 32 + OUT_DIM] = \
            (0.5 * w2t[k * 128:(k + 1) * 128]).astype(FP8_NP)
    w2_eff = np.empty((HID, OUT_DIM), dtype=np.float32)
    for k in range(4):
        w2_eff[k * 128:(k + 1) * 128] = \
            2.0 * W2pT[:, k * 32:k * 32 + OUT_DIM].astype(np.float32)
    r2 = w2_eff.sum(axis=0)                           # [20]
    # q2/q3 weights: k0-2 as W2pT (Sign encode), k3 unscaled ({0,1} encode
    # from the DVE threshold); fold excludes k3
    W2pB = W2pT.copy()
    W2pB[:, 3 * 32:3 * 32 + OUT_DIM] = w2t[384:512].astype(FP8_NP)
    r2_012 = w2_eff[:384].sum(axis=0)                 # [20]
    g = (1.0 - ALPHA ** (np.arange(T, dtype=np.float64) + 1)) / (1.0 - ALPHA)
    theta2 = (THETA - 0.5 * np.outer(r2, g)).astype(np.float32)   # [20, T]
    T2h = np.full((128, 512), 3e38, dtype=BF16_NP)
    T2cP = np.full((128, 1), 3e38, dtype=np.float32)
    T2cPB = np.full((128, 1), 3e38, dtype=np.float32)
    ginf = 1.0 / (1.0 - ALPHA)
    for b in range(B_PER):
        T2h[32 * b:32 * b + OUT_DIM] = theta2[:, :512]
        T2cP[32 * b:32 * b + OUT_DIM, 0] = THETA - 0.5 * r2 * ginf
        T2cPB[32 * b:32 * b + OUT_DIM, 0] = THETA - 0.5 * r2_012 * ginf

    # branch-2: Wl2 fp8 scaled 0.5 (Sign-encoded l1) + T2b fold tile.
    wl2t = Wl2.T.astype(np.float32)                   # [512, 20]
    Wl2pT = np.zeros((128, 4 * OUT_DIM), dtype=FP8_NP)
    for k in range(4):
        Wl2pT[:, k * OUT_DIM:(k + 1) * OUT_DIM] = \
            (0.5 * wl2t[k * 128:(k + 1) * 128]).astype(FP8_NP)
    wl2_eff = np.empty((HID, OUT_DIM), dtype=np.float32)
    for k in range(4):
        wl2_eff[k * 128:(k + 1) * 128] = \
            2.0 * Wl2pT[:, k * OUT_DIM:(k + 1) * OUT_DIM].astype(np.float32)
    rl2 = wl2_eff.sum(axis=0)
    gc = (1.0 - ALPHA ** (np.arange(CP, dtype=np.float64) + 1)) / (1.0 - ALPHA)
    t2b_pat = (THETA - 0.5 * np.outer(rl2, gc)).astype(np.float32)  # [20,156]
    T2b = np.full((128, NB2), 3e38, dtype=BF16_NP)
    T2b[:OUT_DIM] = np.tile(t2b_pat, (1, B_PER))

    # Wl1T [128, KT*HID]: [p, k*HID+o] = Wl1[o, 128k+p]
    Wl1T = np.ascontiguousarray(
        Wl1.T.reshape(KT, 128, HID).transpose(1, 0, 2).reshape(128, KT * HID)
    ).astype(FP8_NP)
    return {"w1s": w1s, "W2pT": W2pT, "W2pB": W2pB, "Wl2pT": Wl2pT,
            "Wl1T": Wl1T, "T2h": T2h, "T2cP": T2cP, "T2cPB": T2cPB,
            "T2b": T2b}


def make_in_maps(spike_input, W1, W2, Wl1, Wl2, perm):
    si = np.asarray(spike_input, dtype=np.float32).reshape(B, C_IN, T)
    perm = np.asarray(perm).astype(np.int64)
    sip = si[:, perm, :]                              # perm-gather (layout only)
    shared = prep_shared_inputs(np.asarray(W1, np.float32),
                                np.asarray(W2, np.float32),
                                np.asarray(Wl1, np.float32),
                                np.asarray(Wl2, np.float32))
    in_maps = []
    for core in range(N_CORES):
        m = dict(shared)
        m.update(prep_core_inputs(si, sip, core))
        in_maps.append(m)
    return in_maps


_IN_SPECS = {
    "siA": ((128, B_PER * T), FP8),
    "siB": ((128, T), FP8),
    "sipT": ((128, KT * NB2), FP8),
    "w1s": ((128, 16 * 256), FP8),
    "W2pT": ((128, 4 * 32), FP8),
    "W2pB": ((128, 4 * 32), FP8),
    "T2cPB": ((128, 1), F32),
    "Wl1T": ((128, KT * HID), FP8),
    "Wl2pT": ((128, 4 * OUT_DIM), FP8),
    "T2h": ((128, 512), BF16),
    "T2cP": ((128, 1), F32),
    "T2b": ((128, NB2), BF16),
}


def build_bass():
    nc = bacc.Bacc("TRN2", target_bir_lowering=False, debug=False)
    ins = {}
    for name, (shape, dt) in _IN_SPECS.items():
        h = nc.dram_tensor(name, list(shape), dt, kind="ExternalInput")
        ins[name] = h[:]
    out_h = nc.dram_tensor("out", [B_PER, 32, T + CP], FP8,
                           kind="ExternalOutput")
    outs = {"out": out_h[:]}
    with tile_mod.TileContext(nc) as tc:
        build_program(tc, outs, ins)
    nc.compile()
    return nc


_NC_CACHE = None


def run(inputs, trace=False, **kw):
    """Run on the 8 NeuronCores; returns (full_output, BassKernelResults)."""
    global _NC_CACHE
    if _NC_CACHE is None:
        _NC_CACHE = build_bass()
    nc = _NC_CACHE
    in_maps = make_in_maps(**inputs)
    res = run_bass_kernel_spmd(nc, in_maps, core_ids=list(range(N_CORES)),
                               trace=trace, **kw)
    parts = []
    for c in range(N_CORES):
        raw = res.results[c]["out"][:, :OUT_DIM, :].astype(np.float32)
        dec = np.empty_like(raw)
        dec[:] = (raw > 0.5)                              # all DVE is_ge {0,1}
        parts.append(dec)
    full = np.concatenate(parts, axis=0).reshape(B, OUT_DIM, 1, 1, T + CP)
    return np.ascontiguousarray(full.astype(np.float32)), res


def kernel(**inputs):
    out, _ = run(inputs)
    return out


# revision 37
# speedup vs baseline: 1.1852x; 1.0624x over previous
"""Trainium2 Bass kernel for nn_LocationSlayerRandom (SLAYER two-branch spiking net).

Contract: kernel(**inputs) takes the FULL unsharded inputs
  spike_input [32,156,1,1,2048] f32, W1 [512,156], W2 [20,512],
  Wl1 [512,2048], Wl2 [20,512], perm [156] i32
and returns the FULL output [32,20,1,1,2204] f32.

Strategy (8 cores, data-parallel over batch, 4 samples/core):

Branch 1 (per sample b):  u1 = psp_t(W1 @ si) = W1 @ psp_t(si)
  - psp_t(si): 5 DVE scans into one 5-plane fp8 tile psAB [128, 5, T]
    (planes 0-3 = channels 0:127 per sample, plane 4 = channels 128:155 of
    all 4 samples packed at partition offsets 32b).
  - fc1 as fp8 DoubleRow: one pass contracts plane b + plane 4 (256 rows)
    against host-stacked weights (plane 1 zero-masked per sample), full
    157 TF/s rate. Thresholds on ACT as Sign(u1-10) -> fp8 {-1,0,1}; fc2
    weights pre-scaled 0.5 and the affine 0.5*rowsum(W2) correction folded
    into the threshold: for t<512 a bf16 T2 tile (DVE compare), for t>=512
    the fold is time-constant to <1e-6 so it becomes a per-partition ACT
    bias. fc2-b1 packs the 4 samples into the 4 PE column groups
    (tile_position=(0,32b)), quartered along t with chained vs scans and
    per-chunk output DMA to shrink the tail.

Branch 2: ul1 = psp_c'(Wl1 @ x_tp), x_tp host-gathered+transposed (layout
  only). A1 on PE in fp8 DoubleRow (plane-strided k-pairs), psp_c' as a
  free-dim DVE scan with per-sample reset pattern, l1 = Sign(ul1-10) on
  ACT, fc2-b2 in fp8 with 0.5-scaled Wl2 and a bf16 T2b compare (the fold
  never saturates within a 156-sample so it stays a full tile).

Numerics: all matmuls fp8 (f32 accumulate). True layer-2 potentials sit
>=7.0 (branch 1) / >=8.2 (branch 2) BELOW their thresholds under this
quantization, so layer-1 bit flips from fp8 cannot flip any output bit.
Outputs are fp8 encodings decoded on host (Sign regions via >=0).
"""

from contextlib import ExitStack

import numpy as np
import ml_dtypes

import concourse.bass as bass
import concourse.mybir as mybir
from concourse import bacc
from concourse import tile as tile_mod
from concourse.bass_utils import run_bass_kernel_spmd

F32 = mybir.dt.float32
BF16 = mybir.dt.bfloat16
FP8 = mybir.dt.float8e4
AL = mybir.AluOpType
AF = mybir.ActivationFunctionType
DR = mybir.MatmulPerfMode.DoubleRow
BF16_NP = ml_dtypes.bfloat16
FP8_NP = ml_dtypes.float8_e4m3

B, C_IN, T = 32, 156, 2048
HID, OUT_DIM = 512, 20
CP = 156                      # permuted taxel axis (branch-2 "time")
N_CORES = 8
B_PER = B // N_CORES          # 4 samples per core
ALPHA = float(np.exp(-1.0 / 10.0))
THETA = 10.0
NB2 = B_PER * CP              # 624, branch-2 packed free dim
KT = T // 128                 # 16 k-tiles over t


def build_program(tc, outs, ins):
    nc = tc.nc
    out = outs["out"]

    with ExitStack() as ctx:
        consts = ctx.enter_context(tc.tile_pool(name="consts", bufs=1))
        work = ctx.enter_context(tc.tile_pool(name="work", bufs=1))
        sgp = ctx.enter_context(tc.tile_pool(name="sgp", bufs=4))
        mid = ctx.enter_context(tc.tile_pool(name="mid", bufs=8))
        psA = ctx.enter_context(tc.tile_pool(name="psA", bufs=2, space="PSUM"))
        psF = ctx.enter_context(tc.tile_pool(name="psF", bufs=2, space="PSUM"))

        # ---------------- constant patterns (gpsimd; SBUF only) ----------
        alpha_t = consts.tile([128, T], F32, tag="alpha")
        nc.gpsimd.memset(alpha_t[:], ALPHA)
        pat624 = consts.tile([128, NB2], F32, tag="pat624")
        nc.gpsimd.memset(pat624[:], ALPHA)
        for j in range(B_PER):
            nc.gpsimd.memset(pat624[:, j * CP:j * CP + 1], 0.0)
        bias_m10 = consts.tile([128, 1], F32, tag="bm10")
        nc.gpsimd.memset(bias_m10[:], -THETA)
        act_warm = consts.tile([128, 1], F32, tag="actwarm")
        nc.scalar.activation(act_warm[:], bias_m10[:], AF.Sign,
                             bias=bias_m10[:])

        # ---------------- inputs (priority-ordered DMAs) -----------------
        siA = consts.tile([128, B_PER * T], FP8, tag="siA")
        siB = consts.tile([128, T], FP8, tag="siB")
        nc.sync.dma_start(siA[:, 0:T], ins["siA"][:, 0:T])
        nc.sync.dma_start(siB[:], ins["siB"][:])
        # sip/wl1 in per-k-pair chunks so A1 pairs start as data lands
        wl1 = consts.tile([128, KT * HID], FP8, tag="wl1")
        sip = consts.tile([128, KT * NB2], FP8, tag="sip")
        for i in range(KT // 2):
            ssl = slice(2 * i * NB2, (2 * i + 2) * NB2)
            wsl = slice(2 * i * HID, (2 * i + 2) * HID)
            nc.sync.dma_start(sip[:, ssl], ins["sipT"][:, ssl])
            nc.sync.dma_start(wl1[:, wsl], ins["Wl1T"][:, wsl])
            if i == 0:
                w1s = consts.tile([128, 16 * 256], FP8, tag="w1s")
                nc.sync.dma_start(w1s[:], ins["w1s"][:])
        for b in range(1, B_PER):
            nc.sync.dma_start(siA[:, b * T:(b + 1) * T],
                              ins["siA"][:, b * T:(b + 1) * T])
        w2p = consts.tile([128, 4 * 32], FP8, tag="w2p")
        nc.sync.dma_start(w2p[:], ins["W2pT"][:])
        w2pb = consts.tile([128, 4 * 32], FP8, tag="w2pb")
        nc.sync.dma_start(w2pb[:], ins["W2pB"][:])
        t2cpb = consts.tile([128, 1], F32, tag="t2cpb")
        nc.sync.dma_start(t2cpb[:], ins["T2cPB"][:])
        wl2p = consts.tile([128, 4 * OUT_DIM], FP8, tag="wl2p")
        nc.sync.dma_start(wl2p[:], ins["Wl2pT"][:])
        t2h = consts.tile([128, 512], BF16, tag="t2h")
        nc.sync.dma_start(t2h[:], ins["T2h"][:])
        t2cp = consts.tile([128, 1], F32, tag="t2cp")
        nc.sync.dma_start(t2cp[:], ins["T2cP"][:])
        t2b = consts.tile([128, NB2], BF16, tag="t2b")
        nc.sync.dma_start(t2b[:], ins["T2b"][:])

        # ---------------- branch-1 input psp scans (DVE, fp8) ------------
        psAB = work.tile([128, 5 * T], FP8, tag="psAB")
        psAB3 = psAB[:].rearrange("p (five t) -> p five t", five=5)
        siA3 = siA[:].rearrange("p (b t) -> p b t", b=B_PER)

        scst = work.tile([128, 5], F32, tag="scst")

        def scan_plane(b, half):
            src = siB[:] if b == 4 else siA3[:, b]
            hsl = slice(half * 1024, (half + 1) * 1024)
            if half == 0:
                # stage the h0 carry in f32 (fp8 init APs wedge the DVE)
                nc.vector.tensor_tensor_scan(psAB3[:, b, hsl],
                                             alpha_t[:, hsl], src[:, hsl],
                                             0.0, AL.mult, AL.add)
                nc.vector.tensor_copy(scst[:, b:b + 1],
                                      psAB3[:, b, 1023:1024])
            else:
                nc.vector.tensor_tensor_scan(psAB3[:, b, hsl],
                                             alpha_t[:, hsl], src[:, hsl],
                                             scst[:, b:b + 1],
                                             AL.mult, AL.add)
        scan_plane(0, 0)
        scan_plane(4, 0)

        # ---------------- branch-2 A1 (fp8 DR, ki-pair granularity) ------
        wl1_3d = wl1[:].rearrange("p (k o) -> p k o", o=HID)
        sip_3d = sip[:].rearrange("p (k c) -> p k c", c=NB2)
        ul1 = []
        l1 = []
        a1_psum = {}

        def a1_pair(m, ki):
            if ki == 0:
                a1_psum[m] = psA.tile([128, 1024], F32, tag="pA",
                                      name=f"pa{m}")
            a1 = a1_psum[m][:, :NB2]
            st, sp = (ki == 0), (ki == KT // 2 - 1)
            msl = slice(m * 128, (m + 1) * 128)
            lhs = wl1_3d[:, 2 * ki:2 * ki + 2, msl]
            nc.tensor.matmul(a1[:, 0:512], lhs,
                             sip_3d[:, 2 * ki:2 * ki + 2, 0:512],
                             start=st, stop=sp, perf_mode=DR)
            nc.tensor.matmul(a1[:, 512:NB2], lhs,
                             sip_3d[:, 2 * ki:2 * ki + 2, 512:NB2],
                             start=st, stop=sp, perf_mode=DR)
            if ki == KT // 2 - 1:
                u = mid.tile([128, NB2], F32, tag="ul1", name=f"ul1{m}")
                nc.vector.tensor_tensor_scan(u[:], pat624[:], a1, 0.0,
                                             AL.mult, AL.add)
                ul1.append(u)

        def l1_thresh(m):
            lt = mid.tile([128, NB2], FP8, tag="l1", name=f"l1{m}")
            nc.scalar.activation(lt[:], ul1[m][:], AF.Sign, bias=bias_m10[:])
            l1.append(lt)

        # ---------------- branch-1 fc1 (fp8 DR) + ACT thresholds ---------
        w1s4 = w1s[:].rearrange("p (bm two h) -> p bm two h", two=2, h=128)
        sg = {}
        for b in range(B_PER):
            sgt = sgp.tile([128, 4 * T], FP8, tag="sg", name=f"sg{b}")
            sg[b] = sgt[:].rearrange("p (m t) -> p m t", m=4)

        def fc1_unit(b, m, half):
            rhs_planes = psAB3[:, b:5:4 - b]
            lhsT = w1s4[:, b * 4 + m]
            pu = psF.tile([128, 1024], F32, tag="pF", name=f"pu{b}{m}{half}")
            for ch in range(2):
                tsl = slice(half * 1024 + ch * 512,
                            half * 1024 + (ch + 1) * 512)
                nc.tensor.matmul(pu[:, ch * 512:(ch + 1) * 512],
                                 lhsT, rhs_planes[:, :, tsl],
                                 start=True, stop=True, perf_mode=DR)
            hsl = slice(half * 1024, (half + 1) * 1024)
            if m == 3 and half == 1:
                # DVE {0,1} encode: pairs with the unscaled k3 of W2pB and
                # a fold that excludes k3 (DVE has idle here; ACT is the
                # tail-gating engine)
                nc.vector.tensor_scalar(sg[b][:, m, hsl], pu[:], THETA,
                                        None, AL.is_ge)
            else:
                nc.scalar.activation(sg[b][:, m, hsl], pu[:], AF.Sign,
                                     bias=bias_m10[:])

        # ---------------- branch 1 fc2 quarter (emitted interleaved) -----
        vs = work.tile([128, T], F32, tag="vs")
        o1 = work.tile([128, T], FP8, tag="o1")

        def fc2b1_q(q):
            qsl = slice(q * 512, (q + 1) * 512)
            puf = psF.tile([128, 1024], F32, tag="pF", name=f"pu2{q}")
            puq = puf[:, 0:512]
            wq = w2p if q < 2 else w2pb
            for k in range(4):
                ksl = slice(k * 32, k * 32 + 32)
                for b in range(B_PER):
                    nc.tensor.matmul(puq[32 * b:32 * b + 32, :],
                                     wq[:, ksl], sg[b][:, k, qsl],
                                     start=(k == 0), stop=(k == 3),
                                     tile_position=(0, 32 * b),
                                     skip_group_check=True)
            init = 0.0 if q == 0 else vs[:, q * 512 - 1:q * 512]
            nc.vector.tensor_tensor_scan(vs[:, qsl], alpha_t[:, 0:512],
                                         puq[:], init, AL.mult, AL.add)
            if q == 0:
                nc.vector.tensor_tensor(o1[:, qsl], vs[:, qsl], t2h[:],
                                        AL.is_ge)
            else:
                tq = t2cp if q == 1 else t2cpb
                nc.vector.tensor_scalar(o1[:, qsl], vs[:, qsl], tq[:],
                                        None, AL.is_ge)
            nc.sync.dma_start(
                out[:, :, qsl].rearrange("b j t -> (b j) t"), o1[:, qsl])

        def fc2b2():
            pl2f = psA.tile([128, 1024], F32, tag="pA", name="pl2")
            pl2 = pl2f[:OUT_DIM, :NB2]
            for k in range(4):
                st, sp = (k == 0), (k == 3)
                ksl = slice(k * OUT_DIM, (k + 1) * OUT_DIM)
                nc.tensor.matmul(pl2[:, 0:512], wl2p[:, ksl],
                                 l1[k][:, 0:512], start=st, stop=sp)
                nc.tensor.matmul(pl2[:, 512:NB2], wl2p[:, ksl],
                                 l1[k][:, 512:NB2], start=st, stop=sp)
            ul2 = mid.tile([128, NB2], F32, tag="ul2")
            nc.vector.tensor_tensor_scan(ul2[:OUT_DIM], pat624[:OUT_DIM],
                                         pl2, 0.0, AL.mult, AL.add)
            o2 = mid.tile([128, NB2], FP8, tag="o2")
            nc.vector.tensor_tensor(o2[:OUT_DIM], ul2[:OUT_DIM],
                                    t2b[:OUT_DIM], AL.is_ge)
            nc.sync.dma_start(
                out[:, :OUT_DIM, T:T + CP].rearrange("b o c -> o b c"),
                o2[:OUT_DIM, :].rearrange("o (b c) -> o b c", c=CP))

        # Emission: h0-major fc1 units (so fc2-b1 q0/q1 unblock mid-kernel)
        # with A1 ki-pairs interleaved one per unit to keep PE streaming
        # without starving the ACT threshold drain.
        pairs = [(m, ki) for m in range(4) for ki in range(KT // 2)]
        pi = 0

        l1_pending = []

        def a1_next(n):
            nonlocal pi
            for _ in range(n):
                if pi < len(pairs):
                    m, ki = pairs[pi]
                    a1_pair(m, ki)
                    if ki == KT // 2 - 1:
                        l1_pending.append(m)
                    pi += 1

        a1_next(8)                      # all of A1-m0 fills the ramp, so
        per_b = {0: 2, 1: 2, 2: 1, 3: 0}    # ul1-0 never stalls the scans
        for b in range(B_PER):              # front-loaded into the early
            if b >= 1:                      # scan-bound windows
                scan_plane(b, 0)            # h0 of all samples first: the
            for m in range(4):              # threshold stream never waits
                fc1_unit(b, m, 0)           # more than one scan chunk
                a1_next(per_b[b])
        scan_plane(0, 1)
        scan_plane(4, 1)
        a1_next(99)
        for b in range(B_PER):
            if b >= 1:
                scan_plane(b, 1)
            if b == 2:
                fc2b1_q(0)                  # h0-only quarters slot into the
            if b == 3:
                fc2b1_q(1)                  # h1 phase once h0 thr are done
                for m in l1_pending:
                    l1_thresh(m)
            for m in range(4):
                fc1_unit(b, m, 1)
        fc2b2()
        fc2b1_q(2)
        fc2b1_q(3)



# ======================= host-side preparation =======================

def prep_core_inputs(si, sip, core):
    """Per-core data tensors (pure layout/dtype prep).
    si/sip are [32,156,2048] f32 (sip already perm-gathered)."""
    sl = si[core * B_PER:(core + 1) * B_PER]          # [4,156,2048]
    siA = np.ascontiguousarray(
        sl[:, :128, :].transpose(1, 0, 2).reshape(128, B_PER * T)
    ).astype(FP8_NP)
    siB = np.zeros((128, T), dtype=FP8_NP)
    for b in range(B_PER):
        siB[32 * b:32 * b + (C_IN - 128)] = sl[b, 128:C_IN, :]
    sp = sip[core * B_PER:(core + 1) * B_PER]         # [4,156,2048]
    sipT = np.ascontiguousarray(
        sp.transpose(2, 0, 1).reshape(KT, 128, NB2)
        .transpose(1, 0, 2).reshape(128, KT * NB2)
    ).astype(FP8_NP)
    return {"siA": siA, "siB": siB, "sipT": sipT}


def prep_shared_inputs(W1, W2, Wl1, Wl2):
    """Weight layouts + threshold tensors, shared by all cores."""
    w1t = np.zeros((160, HID), dtype=np.float32)
    w1t[:C_IN] = W1.T
    # fc1 DR stacked weights [128, (b,m), 2, 128]: plane 0 = channels 0:128,
    # plane 1 = channels 128:156 masked to sample b's psB partitions.
    w1s = np.zeros((128, 16, 2, 128), dtype=FP8_NP)
    for b in range(B_PER):
        for m in range(4):
            w1s[:, b * 4 + m, 0, :] = w1t[:128, m * 128:(m + 1) * 128]
            w1s[32 * b:32 * b + 32, b * 4 + m, 1, :] = \
                w1t[128:160, m * 128:(m + 1) * 128]
    w1s = np.ascontiguousarray(w1s.reshape(128, 16 * 256))

    # fc2 weights fp8, all k scaled 0.5 (uniform ACT Sign encoding),
    # padded to 32 cols per k-tile. [128, 4*32]: [p, k*32+o]
    w2t = W2.T.astype(np.float32)                     # [512, 20]
    W2pT = np.zeros((128, 4 * 32), dtype=FP8_NP)
    for k in range(4):
        W2pT[:, k * 32:k * 32 + OUT_DIM] = \
            (0.5 * w2t[k * 128:(k + 1) * 128]).astype(FP8_NP)
    w2_eff = np.empty((HID, OUT_DIM), dtype=np.float32)
    for k in range(4):
        w2_eff[k * 128:(k + 1) * 128] = \
            2.0 * W2pT[:, k * 32:k * 32 + OUT_DIM].astype(np.float32)
    r2 = w2_eff.sum(axis=0)                           # [20]
    # q2/q3 weights: k0-2 as W2pT (Sign encode), k3 unscaled ({0,1} encode
    # from the DVE threshold); fold excludes k3
    W2pB = W2pT.copy()
    W2pB[:, 3 * 32:3 * 32 + OUT_DIM] = w2t[384:512].astype(FP8_NP)
    r2_012 = w2_eff[:384].sum(axis=0)                 # [20]
    g = (1.0 - ALPHA ** (np.arange(T, dtype=np.float64) + 1)) / (1.0 - ALPHA)
    theta2 = (THETA - 0.5 * np.outer(r2, g)).astype(np.float32)   # [20, T]
    T2h = np.full((128, 512), 3e38, dtype=BF16_NP)
    T2cP = np.full((128, 1), 3e38, dtype=np.float32)
    T2cPB = np.full((128, 1), 3e38, dtype=np.float32)
    ginf = 1.0 / (1.0 - ALPHA)
    for b in range(B_PER):
        T2h[32 * b:32 * b + OUT_DIM] = theta2[:, :512]
        T2cP[32 * b:32 * b + OUT_DIM, 0] = THETA - 0.5 * r2 * ginf
        T2cPB[32 * b:32 * b + OUT_DIM, 0] = THETA - 0.5 * r2_012 * ginf

    # branch-2: Wl2 fp8 scaled 0.5 (Sign-encoded l1) + T2b fold tile.
    wl2t = Wl2.T.astype(np.float32)                   # [512, 20]
    Wl2pT = np.zeros((128, 4 * OUT_DIM), dtype=FP8_NP)
    for k in range(4):
        Wl2pT[:, k * OUT_DIM:(k + 1) * OUT_DIM] = \
            (0.5 * wl2t[k * 128:(k + 1) * 128]).astype(FP8_NP)
    wl2_eff = np.empty((HID, OUT_DIM), dtype=np.float32)
    for k in range(4):
        wl2_eff[k * 128:(k + 1) * 128] = \
            2.0 * Wl2pT[:, k * OUT_DIM:(k + 1) * OUT_DIM].astype(np.float32)
    rl2 = wl2_eff.sum(axis=0)
    gc = (1.0 - ALPHA ** (np.arange(CP, dtype=np.float64) + 1)) / (1.0 - ALPHA)
    t2b_pat = (THETA - 0.5 * np.outer(rl2, gc)).astype(np.float32)  # [20,156]
    T2b = np.full((128, NB2), 3e38, dtype=BF16_NP)
    T2b[:OUT_DIM] = np.tile(t2b_pat, (1, B_PER))

    # Wl1T [128, KT*HID]: [p, k*HID+o] = Wl1[o, 128k+p]
    Wl1T = np.ascontiguousarray(
        Wl1.T.reshape(KT, 128, HID).transpose(1, 0, 2).reshape(128, KT * HID)
    ).astype(FP8_NP)
    return {"w1s": w1s, "W2pT": W2pT, "W2pB": W2pB, "Wl2pT": Wl2pT,
            "Wl1T": Wl1T, "T2h": T2h, "T2cP": T2cP, "T2cPB": T2cPB,
            "T2b": T2b}


def make_in_maps(spike_input, W1, W2, Wl1, Wl2, perm):
    si = np.asarray(spike_input, dtype=np.float32).reshape(B, C_IN, T)
    perm = np.asarray(perm).astype(np.int64)
    sip = si[:, perm, :]                              # perm-gather (layout only)
    shared = prep_shared_inputs(np.asarray(W1, np.float32),
                                np.asarray(W2, np.float32),
                                np.asarray(Wl1, np.float32),
                                np.asarray(Wl2, np.float32))
    in_maps = []
    for core in range(N_CORES):
        m = dict(shared)
        m.update(prep_core_inputs(si, sip, core))
        in_maps.append(m)
    return in_maps


_IN_SPECS = {
    "siA": ((128, B_PER * T), FP8),
    "siB": ((128, T), FP8),
    "sipT": ((128, KT * NB2), FP8),
    "w1s": ((128, 16 * 256), FP8),
    "W2pT": ((128, 4 * 32), FP8),
    "W2pB": ((128, 4 * 32), FP8),
    "T2cPB": ((128, 1), F32),
    "Wl1T": ((128, KT * HID), FP8),
    "Wl2pT": ((128, 4 * OUT_DIM), FP8),
    "T2h": ((128, 512), BF16),
    "T2cP": ((128, 1), F32),
    "T2b": ((128, NB2), BF16),
}


def build_bass():
    nc = bacc.Bacc("TRN2", target_bir_lowering=False, debug=False)
    ins = {}
    for name, (shape, dt) in _IN_SPECS.items():
        h = nc.dram_tensor(name, list(shape), dt, kind="ExternalInput")
        ins[name] = h[:]
    out_h = nc.dram_tensor("out", [B_PER, 32, T + CP], FP8,
                           kind="ExternalOutput")
    outs = {"out": out_h[:]}
    with tile_mod.TileContext(nc) as tc:
        build_program(tc, outs, ins)
    nc.compile()
    return nc


_NC_CACHE = None


def run(inputs, trace=False, **kw):
    """Run on the 8 NeuronCores; returns (full_output, BassKernelResults)."""
    global _NC_CACHE
    if _NC_CACHE is None:
        _NC_CACHE = build_bass()
    nc = _NC_CACHE
    in_maps = make_in_maps(**inputs)
    res = run_bass_kernel_spmd(nc, in_maps, core_ids=list(range(N_CORES)),
                               trace=trace, **kw)
    parts = []
    for c in range(N_CORES):
        raw = res.results[c]["out"][:, :OUT_DIM, :].astype(np.float32)
        dec = np.empty_like(raw)
        dec[:] = (raw > 0.5)                              # all DVE is_ge {0,1}
        parts.append(dec)
    full = np.concatenate(parts, axis=0).reshape(B, OUT_DIM, 1, 1, T + CP)
    return np.ascontiguousarray(full.astype(np.float32)), res


def kernel(**inputs):
    out, _ = run(inputs)
    return out
